# revision 1
# baseline (speedup 1.0000x reference)
"""Multi-head causal attention (B=2, T=2048, C=4096, H=32) on 8 Trainium2
NeuronCores, tensor-parallel over heads (Megatron-style).

Per core m (4 heads each):
  phase 1: q/k/v projections from full x (weights column-sharded,
           host-pre-transposed into lhsT/rhs layouts). RoPE applied to q/k
           at PSUM eviction (all rotary freqs == 1.0 in this model, so
           cos/sin are per-position scalars; head_dim is host-permuted to
           [evens, odds] so rotation pairs sit in partition halves; the
           half-swap runs through SBUF->SBUF DMA).
  phase 2: attention per (head, batch) with scores computed TRANSPOSED
           [k, q]: u = exp(scale * sT) (no max subtraction needed at these
           scales), causal-masked; o.T = v.T @ probs.T accumulates in PSUM;
           the softmax denominator accumulates via an all-ones stationary
           matmul; normalization at eviction.
  phase 3: AllToAll redistributes o.T so each core owns ALL heads for its
           row-slice; y_rows = a_rows @ wo.T with the full wo.
Host gathers the 8 row-slices. Host does layout prep (transpose/cast) and
the final concatenate only.
"""

import os
import sys

import numpy as np

for _p in ("/opt/trn_rl_repo", "/root/.axon_site/_ro/trn_rl_repo"):
    if os.path.isdir(_p) and _p not in sys.path:
        sys.path.insert(0, _p)

import ml_dtypes

import concourse.bacc as bacc
import concourse.bass as bass
import concourse.mybir as mybir
import concourse.tile as tile
from concourse.bass_utils import run_bass_kernel_spmd

BF16 = ml_dtypes.bfloat16
P = 128
NCORES = 8
DT = mybir.dt.bfloat16
F32 = mybir.dt.float32
ActFn = mybir.ActivationFunctionType

FULL = dict(B=2, T=2048, C=4096, H=32, W=512, QT=512)


def _dims(cfg):
    B, T, C, H = cfg["B"], cfg["T"], cfg["C"], cfg["H"]
    W, QT = cfg["W"], cfg["QT"]
    HD = C // H
    assert HD == P
    HL = H // NCORES
    R = B * T
    RS = R // NCORES
    KO = C // P
    assert R % W == 0 and T % QT == 0 and QT % P == 0 and W % P == 0
    assert RS == QT * B * (T // QT) // NCORES or True
    return B, T, C, H, HD, HL, R, RS, KO, W, QT


def build_nc(cfg=FULL, big_dma_engine="gpsimd"):
    B, T, C, H, HD, HL, R, RS, KO, W, QT = _dims(cfg)
    NW = R // W
    NKT = T // P
    SCALE = float(HD) ** -0.5
    MOFF = QT - P  # max diagonal offset in the causal mask table

    nc = bacc.Bacc(None, num_devices=NCORES)
    big_dma = getattr(nc, big_dma_engine).dma_start

    xT = nc.dram_tensor("xT", [P, KO, R], DT, kind="ExternalInput")
    wqT = nc.dram_tensor("wqT", [P, KO, HL * HD], DT, kind="ExternalInput")
    wkT = nc.dram_tensor("wkT", [P, KO, HL * HD], DT, kind="ExternalInput")
    wvT = nc.dram_tensor("wvT", [P, KO, HL * HD], DT, kind="ExternalInput")
    woT = nc.dram_tensor("woT", [P, KO, C], DT, kind="ExternalInput")
    cosR = nc.dram_tensor("cosR", [P, R], DT, kind="ExternalInput")
    sinS = nc.dram_tensor("sinS", [P, R], DT, kind="ExternalInput")
    maskb = nc.dram_tensor("maskb", [P, MOFF + QT], DT, kind="ExternalInput")
    y = nc.dram_tensor("y", [RS, C], F32, kind="ExternalOutput")

    qT_d = nc.dram_tensor("qT_d", [P, HL, R], DT)
    kT_d = nc.dram_tensor("kT_d", [P, HL, R], DT)
    v_d = nc.dram_tensor("v_d", [P, R // P, HL * HD], DT)
    a2a_i = nc.dram_tensor("a2a_i", [NCORES, HL * HD, RS], DT)
    a2a_o = nc.dram_tensor("a2a_o", [NCORES, HL * HD, RS], DT)

    with tile.TileContext(nc) as tc:
        # ---------------- phase 1: q/k/v projections + rope ----------------
        with (
            tc.tile_pool(name="wp", bufs=1) as wp,
            tc.tile_pool(name="tab1", bufs=1) as tab1,
            tc.tile_pool(name="xp", bufs=2) as xp,
            tc.tile_pool(name="ev1", bufs=3) as ev1,
            tc.tile_pool(name="ps1", bufs=2, space="PSUM") as ps1,
        ):
            wq_sb = wp.tile([P, KO, HL * HD], DT, tag="wq")
            wk_sb = wp.tile([P, KO, HL * HD], DT, tag="wk")
            wv_sb = wp.tile([P, KO, HL * HD], DT, tag="wv")
            big_dma(wq_sb[:], wqT[:])
            big_dma(wk_sb[:], wkT[:])
            big_dma(wv_sb[:], wvT[:])
            cos_sb = tab1.tile([P, R], DT, tag="cos")
            sin_sb = tab1.tile([P, R], DT, tag="sin")
            big_dma(cos_sb[:], cosR[:])
            big_dma(sin_sb[:], sinS[:])

            for w in range(NW):
                xw = xp.tile([P, KO, W], DT, tag="xw")
                big_dma(xw[:], xT[:, :, w * W:(w + 1) * W])
                rsl = slice(w * W, (w + 1) * W)

                for wsb, dst in ((wq_sb, qT_d), (wk_sb, kT_d)):
                    for h in range(HL):
                        pt = ps1.tile([P, W], F32, tag="pqk")
                        for k in range(KO):
                            nc.tensor.matmul(
                                pt[:], wsb[:, k, h * HD:(h + 1) * HD], xw[:, k],
                                start=(k == 0), stop=(k == KO - 1),
                            )
                        # rope: rot = raw*cos + swap(raw)*sinS (sign-split sin);
                        # engines need same-start-partition operands, so the
                        # half-swap goes through SBUF->SBUF DMA.
                        raw = ev1.tile([P, W], DT, tag="raw")
                        nc.scalar.activation(raw[:], pt[:], ActFn.Copy)
                        sw = ev1.tile([P, W], DT, tag="sw")
                        nc.sync.dma_start(sw[0:64, :], raw[64:128, :])
                        nc.sync.dma_start(sw[64:128, :], raw[0:64, :])
                        t1 = ev1.tile([P, W], DT, tag="t1")
                        nc.vector.tensor_tensor(
                            t1[:], sw[:], sin_sb[:, rsl], mybir.AluOpType.mult)
                        rot = ev1.tile([P, W], DT, tag="rot")
                        nc.vector.tensor_tensor(
                            rot[:], raw[:], cos_sb[:, rsl], mybir.AluOpType.mult)
                        nc.vector.tensor_tensor(
                            rot[:], rot[:], t1[:], mybir.AluOpType.add)
                        nc.sync.dma_start(dst[:, h, rsl], rot[:])

                for rs_ in range(W // P):
                    pt = ps1.tile([P, HL * HD], F32, tag="pv")
                    for k in range(KO):
                        nc.tensor.matmul(
                            pt[:], xw[:, k, rs_ * P:(rs_ + 1) * P], wv_sb[:, k],
                            start=(k == 0), stop=(k == KO - 1),
                        )
                    vv = ev1.tile([P, HL * HD], DT, tag="vv")
                    nc.scalar.activation(vv[:], pt[:], ActFn.Copy)
                    nc.sync.dma_start(v_d[:, w * (W // P) + rs_, :], vv[:])

        # ---------------- phase 2: attention ----------------
        with (
            tc.tile_pool(name="tab2", bufs=1) as tab2,
            tc.tile_pool(name="att", bufs=2) as att,
            tc.tile_pool(name="up", bufs=3) as up,
            tc.tile_pool(name="ps2", bufs=2, space="PSUM") as ps2,
        ):
            ones_sb = tab2.tile([P, P], DT, tag="ones")
            nc.vector.memset(ones_sb[:], 1.0)
            mask_sb = tab2.tile([P, MOFF + QT], DT, tag="mask")
            nc.sync.dma_start(mask_sb[:], maskb[:])

            for b in range(B):
                vb = att.tile([P, NKT, HL * HD], DT, tag="vb")
                big_dma(vb[:], v_d[:, b * NKT:(b + 1) * NKT, :])
                for h in range(HL):
                    kTb = att.tile([P, T], DT, tag="kTb")
                    big_dma(kTb[:], kT_d[:, h, b * T:(b + 1) * T])
                    for qt in range(T // QT):
                        qTt = att.tile([P, QT], DT, tag="qTt")
                        nc.sync.dma_start(
                            qTt[:], qT_d[:, h, b * T + qt * QT: b * T + (qt + 1) * QT])
                        po = ps2.tile([P, QT], F32, tag="po")
                        pd = ps2.tile([P, QT], F32, tag="pd")
                        nkt = (qt + 1) * (QT // P)
                        for kt in range(nkt):
                            pS = ps2.tile([P, QT], F32, tag="pS")
                            nc.tensor.matmul(
                                pS[:], kTb[:, kt * P:(kt + 1) * P], qTt[:],
                                start=True, stop=True,
                            )
                            u = up.tile([P, QT], DT, tag="u")
                            nc.scalar.activation(u[:], pS[:], ActFn.Exp, scale=SCALE)
                            off = (kt - qt * (QT // P)) * P
                            if off >= 0:  # diagonal block: apply causal mask
                                s = MOFF - off
                                nc.vector.tensor_tensor(
                                    u[:], u[:], mask_sb[:, s:s + QT],
                                    mybir.AluOpType.mult)
                            first, last = (kt == 0), (kt == nkt - 1)
                            nc.tensor.matmul(
                                po[:], vb[:, kt, h * HD:(h + 1) * HD], u[:],
                                start=first, stop=last)
                            nc.tensor.matmul(
                                pd[:], ones_sb[:], u[:], start=first, stop=last)
                        rec = up.tile([P, QT], F32, tag="rec")
                        nc.vector.reciprocal(rec[:], pd[:])
                        ot = up.tile([P, QT], DT, tag="ot")
                        nc.vector.tensor_tensor(
                            ot[:], po[:], rec[:], mybir.AluOpType.mult)
                        gq = b * (T // QT) + qt  # global row-block index
                        dst_core = (gq * QT) // RS
                        roff = (gq * QT) % RS
                        nc.sync.dma_start(
                            a2a_i[dst_core, h * HD:(h + 1) * HD,
                                  roff:roff + QT], ot[:])

        # ---------------- phase 3: all-to-all + output projection ----------
        with (
            tc.tile_pool(name="ap3", bufs=1) as ap3,
            tc.tile_pool(name="wop", bufs=2) as wop,
            tc.tile_pool(name="yp", bufs=3) as yp,
            tc.tile_pool(name="ps3", bufs=2, space="PSUM") as ps3,
        ):
            nc.gpsimd.collective_compute(
                "AllToAll",
                mybir.AluOpType.bypass,
                replica_groups=[list(range(NCORES))],
                ins=[a2a_i[:]],
                outs=[a2a_o[:]],
            )
            aT_sb = ap3.tile([P, KO, RS], DT, tag="aT")
            big_dma(aT_sb[:], a2a_o[:].rearrange("s (i d) r -> d (s i) r", d=P))

            NCB = C // QT
            for cb in range(NCB):
                wot = wop.tile([P, KO, QT], DT, tag="wot")
                big_dma(wot[:], woT[:, :, cb * QT:(cb + 1) * QT])
                for rs_ in range(RS // P):
                    pt = ps3.tile([P, QT], F32, tag="py")
                    for k in range(KO):
                        nc.tensor.matmul(
                            pt[:], aT_sb[:, k, rs_ * P:(rs_ + 1) * P], wot[:, k],
                            start=(k == 0), stop=(k == KO - 1),
                        )
                    yt = yp.tile([P, QT], F32, tag="yt")
                    nc.scalar.activation(yt[:], pt[:], ActFn.Copy)
                    nc.sync.dma_start(
                        y[rs_ * P:(rs_ + 1) * P, cb * QT:(cb + 1) * QT], yt[:])

    nc.compile()
    return nc


def _as_lhsT_tiles(w):
    """[M, K] row-major -> [P, K//P, M]: out[p, ko, m] = w[m, ko*P + p]."""
    M, K = w.shape
    return np.ascontiguousarray(
        w.reshape(M, K // P, P).transpose(2, 1, 0)).astype(BF16)


def prep_inputs(x, wq, wk, wv, wo, cfg=FULL):
    B, T, C, H, HD, HL, R, RS, KO, W, QT = _dims(cfg)
    MOFF = QT - P
    rope_perm = np.concatenate([np.arange(0, HD, 2), np.arange(1, HD, 2)])

    xflat = np.ascontiguousarray(x.reshape(R, C))
    xT = _as_lhsT_tiles(xflat)                       # [P, KO, R]
    woT = _as_lhsT_tiles(wo)                         # [P, KO, C]

    t = (np.arange(R) % T).astype(np.float64)
    cosR = np.broadcast_to(np.cos(t), (P, R)).astype(BF16)
    sin_row = np.sin(t)
    sinS = np.empty((P, R), np.float64)
    sinS[0:64, :] = -sin_row
    sinS[64:128, :] = sin_row
    sinS = sinS.astype(BF16)

    # mask[p, u] = 1 iff u >= p + MOFF; diagonal block with key-offset `off`
    # uses slice [MOFF-off : MOFF-off+QT] giving allowed = (qf >= kp + off)
    uu = np.arange(MOFF + QT)
    maskb = (uu[None, :] >= (np.arange(P)[:, None] + MOFF)).astype(BF16)

    per_core = []
    for m in range(NCORES):
        sl = slice(m * HL * HD, (m + 1) * HL * HD)
        wq_m = wq[sl].reshape(HL, HD, C)[:, rope_perm, :].reshape(HL * HD, C)
        wk_m = wk[sl].reshape(HL, HD, C)[:, rope_perm, :].reshape(HL * HD, C)
        per_core.append(dict(
            xT=xT,
            wqT=_as_lhsT_tiles(wq_m),
            wkT=_as_lhsT_tiles(wk_m),
            wvT=_as_lhsT_tiles(wv[sl]),
            woT=woT,
            cosR=cosR,
            sinS=sinS,
            maskb=maskb,
        ))
    return per_core


_NC_CACHE = None


def kernel(x, wq, wk, wv, wo):
    global _NC_CACHE
    cfg = FULL
    B, T, C = cfg["B"], cfg["T"], cfg["C"]
    if _NC_CACHE is None:
        _NC_CACHE = build_nc(cfg)
    nc = _NC_CACHE
    in_maps = prep_inputs(
        np.asarray(x, np.float32), np.asarray(wq, np.float32),
        np.asarray(wk, np.float32), np.asarray(wv, np.float32),
        np.asarray(wo, np.float32), cfg)
    res = run_bass_kernel_spmd(nc, in_maps, core_ids=list(range(NCORES)))
    y = np.concatenate([r["y"] for r in res.results], axis=0)
    return y.reshape(B, T, C).astype(np.float32)

